# revision 1
# baseline (speedup 1.0000x reference)
"""Trainium2 Bass kernel for nn_Attention_81020263072470.

Math (reference):
    q = queries @ W_q.T                    [B, H]
    k = keys @ W_k.T                       [B, S, H]
    scores = tanh(k + q[:, None, :]) @ w_v [B, S]
    e = exp(scores); att = e / sum(e)      (global sum over all B*S)
    rep = einsum('bsd,bs->bd', keys, att)  [B, D]

Strategy: data-parallel over batch (32 batches -> 4 per core on 8 cores).
The host passes keys pre-transposed [b, d, s] in bf16 - the ONLY keys layout
the device reads (16 MiB/core instead of 32+: below the fp32 memory
roofline). All on-chip transposes of the big tensor are gone; the
TensorEngine runs only the k-projection and score matmuls, while the
otherwise-idle VectorEngine computes the output weighted sum with fused
multiply-reduce (tensor_tensor_reduce) straight from the same bf16 keysT.

Per core, per 512-row tile:
  - k^T chunk [h=128, s=512] = sum_dc WkT[dc].T @ keysT[dc]   (bf16 matmuls)
  - tanh(k + q) via ACT with per-partition bias q^T            (psum->fp32r)
  - score row [1,512] = sum_hc wvT[hc].T @ tanh[hc]            (fp32r)
  - e = exp(score) with fused per-tile sum (ACT accum_out)
  - e_rep [128,512] = ones x e (rank-1 PE matmul), ACT-copied to bf16
  - rep_acc[d,1] per d-chunk += sum_s keysT[d,s]*e_rep[d,s] via DVE
    tensor_tensor_reduce, chained across tiles through the scalar init
The global exp-sum normalization is linear, so each core returns
UNNORMALIZED weighted sums plus partial exp-sums; the host computes the
global scalar sum and divides (replacing the all-reduce).

q = queries @ W_q.T is computed on the host (tiny: 32x512x512, fp32).
"""
import numpy as np
from contextlib import ExitStack

# ---- problem constants (hardcoded per contract) ----
B, S, D, H = 32, 4096, 512, 512
N_CORES = 8
B_SHARD = B // N_CORES          # 4 batches per core
TILE_S = 512                    # s-rows per tile
N_TILES = S // TILE_S           # 8 tiles per batch
N_DC = D // 128                 # 4 d-chunks
N_HC = H // 128                 # 4 h-chunks

_RUNNER = None
_NC = None


def _build_nc(repeat=1):
    import concourse.bacc as bacc
    import concourse.tile as tile
    import concourse.mybir as mybir

    F32 = mybir.dt.float32
    F32R = mybir.dt.float32r
    BF16 = mybir.dt.bfloat16
    AF = mybir.ActivationFunctionType
    ALU = mybir.AluOpType

    nc = bacc.Bacc("TRN2", target_bir_lowering=False, debug=False,
                   num_devices=N_CORES)

    kT_d = nc.dram_tensor("kT_s", [B_SHARD, N_TILES, 128, N_DC, TILE_S], BF16,
                      kind="ExternalInput")
    wkT_d = nc.dram_tensor("wkT", [128, N_DC, H], BF16, kind="ExternalInput")
    wvT_d = nc.dram_tensor("wvT", [128, N_HC], F32R, kind="ExternalInput")
    qT_d = nc.dram_tensor("qT", [128, N_HC, B_SHARD], F32, kind="ExternalInput")
    rep_d = nc.dram_tensor("rep_acc", [128, B_SHARD * N_DC], F32,
                           kind="ExternalOutput")
    esum_d = nc.dram_tensor("esums", [1, B_SHARD * N_TILES], F32,
                            kind="ExternalOutput")

    with ExitStack() as ctx:
        tc = ctx.enter_context(tile.TileContext(nc))
        cpool = ctx.enter_context(tc.tile_pool(name="const", bufs=1))
        p_kT = ctx.enter_context(tc.tile_pool(name="kT", bufs=4))
        p_tanh = ctx.enter_context(tc.tile_pool(name="tanh", bufs=3))
        p_small = ctx.enter_context(tc.tile_pool(name="small", bufs=3))
        ps = ctx.enter_context(tc.tile_pool(name="psum", bufs=1, space="PSUM"))

        wkT = cpool.tile([128, N_DC, H], BF16)
        nc.sync.dma_start(wkT[:], wkT_d[:])
        wvT = cpool.tile([128, N_HC], F32R)
        nc.sync.dma_start(wvT[:], wvT_d[:])
        qT = cpool.tile([128, N_HC, B_SHARD], F32)
        nc.sync.dma_start(qT[:], qT_d[:])

        esums = cpool.tile([1, B_SHARD * N_TILES], F32)
        rep_acc = cpool.tile([128, B_SHARD * N_DC], F32)

        def emit_head(b, t):
            """DMA + main matmuls + tanh for tile (b, t). Returns state."""
            s0 = t * TILE_S
            kT = p_kT.tile([128, N_DC, TILE_S], BF16)
            nc.sync.dma_start(kT[:], kT_d[b, t])
            tanh_sb = p_tanh.tile([128, N_HC, TILE_S], F32R)
            for hc in range(N_HC):
                pk = ps.tile([128, TILE_S], F32, tag="pk", bufs=3)
                for dc in range(N_DC):
                    nc.tensor.matmul(
                        pk[:],
                        wkT[:, dc, hc * 128:(hc + 1) * 128],
                        kT[:, dc],
                        start=(dc == 0), stop=(dc == N_DC - 1))
                nc.scalar.activation(
                    tanh_sb[:, hc], pk[:], AF.Tanh,
                    bias=qT[:, hc, b:b + 1])
            return (b, t, kT, tanh_sb)

        def emit_tail(state):
            """score + exp + broadcast + weighted-sum for a tile."""
            b, t, kT, tanh_sb = state
            ti = b * N_TILES + t
            psc = ps.tile([1, TILE_S], F32, tag="psc", bufs=3)
            for hc in range(N_HC):
                nc.tensor.matmul(
                    psc[:], wvT[:, hc:hc + 1], tanh_sb[:, hc],
                    start=(hc == 0), stop=(hc == N_HC - 1))
            e_sb = p_small.tile([1, TILE_S], BF16, tag="e")
            nc.scalar.activation(e_sb[:], psc[:], AF.Exp,
                                 accum_out=esums[0:1, ti:ti + 1])
            e_rep = p_small.tile([128, TILE_S], BF16, tag="erep")
            nc.gpsimd.partition_broadcast(e_rep[:], e_sb[:])
            scr = p_small.tile([128, TILE_S], BF16, tag="scr")
            rep_t = p_small.tile([128, N_DC], F32, tag="rept")
            for dc in range(N_DC):
                nc.vector.affine_mul_reduce(
                    out=scr[:], accum_out=rep_t[:, dc:dc + 1],
                    in0=kT[:, dc], in1=e_rep[:], scale=1.0, bias=0.0)
            bc = b * N_DC
            if t == 0:
                nc.vector.tensor_copy(rep_acc[:, bc:bc + N_DC], rep_t[:])
            else:
                nc.vector.tensor_add(rep_acc[:, bc:bc + N_DC],
                                     rep_acc[:, bc:bc + N_DC], rep_t[:])

        def emit_head_part(b, t, kT, tanh_sb, hcs):
            for hc in hcs:
                pk = ps.tile([128, TILE_S], F32, tag="pk", bufs=3)
                for dc in range(N_DC):
                    nc.tensor.matmul(
                        pk[:],
                        wkT[:, dc, hc * 128:(hc + 1) * 128],
                        kT[:, dc],
                        start=(dc == 0), stop=(dc == N_DC - 1))
                nc.scalar.activation(
                    tanh_sb[:, hc], pk[:], AF.Tanh,
                    bias=qT[:, hc, b:b + 1])

        for _rep in range(repeat):
            pending = None
            for b in range(B_SHARD):
                for t in range(N_TILES):
                    kT = p_kT.tile([128, N_DC, TILE_S], BF16)
                    nc.sync.dma_start(kT[:], kT_d[b, t])
                    tanh_sb = p_tanh.tile([128, N_HC, TILE_S], F32R)
                    emit_head_part(b, t, kT, tanh_sb, range(0, 2))
                    if pending is not None:
                        emit_tail(pending)
                    emit_head_part(b, t, kT, tanh_sb, range(2, N_HC))
                    pending = (b, t, kT, tanh_sb)
            emit_tail(pending)

        nc.sync.dma_start(rep_d[:], rep_acc[:])
        nc.sync.dma_start(esum_d[:], esums[:])

    nc.compile()
    return nc


def _make_runner(repeat=1):
    """Build the Bass module and return a jitted SPMD callable."""
    import jax
    import numpy as _np
    from jax.sharding import Mesh, PartitionSpec
    from jax.experimental.shard_map import shard_map
    import concourse.mybir as mybir
    from concourse import bass2jax

    bass2jax.install_neuronx_cc_hook()
    global _NC
    nc = _build_nc(repeat)
    if repeat == 1:
        _NC = nc

    partition_name = (nc.partition_id_tensor.name
                      if nc.partition_id_tensor else None)
    in_names, out_names, out_avals, zero_shapes = [], [], [], []
    for alloc in nc.m.functions[0].allocations:
        if not isinstance(alloc, mybir.MemoryLocationSet):
            continue
        name = alloc.memorylocations[0].name
        if alloc.kind == "ExternalInput":
            if name != partition_name:
                in_names.append(name)
        elif alloc.kind == "ExternalOutput":
            shape = tuple(alloc.tensor_shape)
            dtype = mybir.dt.np(alloc.dtype)
            out_names.append(name)
            out_avals.append(jax.core.ShapedArray(shape, dtype))
            zero_shapes.append((shape, dtype))
    n_params = len(in_names)
    all_in_names = list(in_names) + list(out_names)
    if partition_name is not None:
        all_in_names.append(partition_name)

    def _body(*args):
        operands = list(args)
        if partition_name is not None:
            operands.append(bass2jax.partition_id_tensor())
        outs = bass2jax._bass_exec_p.bind(
            *operands,
            out_avals=tuple(out_avals),
            in_names=tuple(all_in_names),
            out_names=tuple(out_names),
            lowering_input_output_aliases=(),
            sim_require_finite=True,
            sim_require_nnan=True,
            nc=nc,
        )
        return tuple(outs)

    devices = jax.devices()[:N_CORES]
    mesh = Mesh(_np.asarray(devices), ("core",))
    n_outs = len(out_names)
    in_specs = (PartitionSpec("core"),) * (n_params + n_outs)
    out_specs = (PartitionSpec("core"),) * n_outs
    sharded = jax.jit(
        shard_map(_body, mesh=mesh, in_specs=in_specs, out_specs=out_specs,
                  check_rep=False),
        donate_argnums=tuple(range(n_params, n_params + n_outs)),
        keep_unused=True,
    )

    def make_zeros():
        return [_np.zeros((N_CORES * s[0], *s[1:]), dt)
                for (s, dt) in zero_shapes]

    return sharded, in_names, out_names, make_zeros, mesh


def _get_runner():
    global _RUNNER
    if _RUNNER is None:
        _RUNNER = _make_runner()
    return _RUNNER


def _prep_inputs(keys, queries, W_k, W_q, w_v):
    """Host-side prep: shard keys, transform small tensors. Returns a dict
    name -> concatenated-along-axis-0 global array (per-core shards)."""
    import ml_dtypes
    BF = ml_dtypes.bfloat16

    keys = np.asarray(keys, dtype=np.float32)
    keysT = np.ascontiguousarray(
        keys.transpose(0, 2, 1)).astype(BF)          # [B, D, S]
    # tile-contiguous layout [B, t, p, dc, s]: elem = keysT[b, dc*128+p, t*T+s]
    keysT = np.ascontiguousarray(
        keysT.reshape(B, N_DC, 128, N_TILES, TILE_S).transpose(0, 3, 2, 1, 4))
    q = (queries.astype(np.float32) @ W_q.astype(np.float32).T)  # [B, H]

    # WkT host layout [128, dc, H]: [p, dc, h] = W_k[h, dc*128+p]
    wkT = np.ascontiguousarray(
        W_k.astype(np.float32).T.reshape(N_DC, 128, H).transpose(1, 0, 2)
    ).astype(BF)
    wvT = np.ascontiguousarray(w_v.astype(np.float32)[0].reshape(N_HC, 128).T)

    ins = {"kT_s": keysT,                         # [B, D, S] (axis0 -> 4/core)
           "wkT": np.tile(wkT, (N_CORES, 1, 1)),  # replicated
           "wvT": np.tile(wvT, (N_CORES, 1))}
    qT_all = []
    for c in range(N_CORES):
        qc = q[c * B_SHARD:(c + 1) * B_SHARD]     # [4, H]
        qT_all.append(qc.T.reshape(N_HC, 128, B_SHARD).transpose(1, 0, 2))
    ins["qT"] = np.ascontiguousarray(np.concatenate(qT_all, axis=0))
    return ins


def kernel(keys, queries, W_k, W_q, w_v):
    sharded, in_names, out_names, make_zeros, _mesh = _get_runner()
    ins = _prep_inputs(keys, queries, W_k, W_q, w_v)
    args = [ins[n] for n in in_names] + make_zeros()
    outs = sharded(*args)
    res = {n: np.asarray(outs[i]) for i, n in enumerate(out_names)}
    # rep_acc: per core [128, b*N_DC + dc] with d = dc*128 + p
    acc = res["rep_acc"].reshape(N_CORES, 128, B_SHARD, N_DC)
    rep_raw = acc.transpose(0, 2, 3, 1).reshape(B, D)   # [b, dc*128+p]
    esum_total = np.float32(res["esums"].astype(np.float64).sum())
    return (rep_raw / esum_total).astype(np.float32)

